# revision 35
# baseline (speedup 1.0000x reference)
"""GCGRU cell (graph-conv GRU, diffusion order 3) on 8 TRN2 NeuronCores.

Data-parallel over the batch dim (512 per core). Per core, activations are
channel-on-partition [C, (b, n)]; the node-dim diffusion transposes 3-batch
groups through the PE transpose datapath (batched same-shape so they
pipeline) and multiplies against a host-precomputed block-diagonal
[M^1 | M^2 | M^3]. The h-diffusion conv chunks (gh1..gh3, ~1% of the
pre-activation variance) run as fp8 DoubleRow matmuls (e4m3 activations x
e5m2 weights, scales 1 so they accumulate straight into the shared f32 PSUM
group with the bf16 chunks); the z chunks stay bf16. sigmoid/tanh run on the
scalar engine out of PSUM with fused bias; copies are spread across the
scalar/vector/gpsimd engines.
"""
import numpy as np
import ml_dtypes

import concourse.bacc as bacc
import concourse.mybir as mybir
from concourse.tile import TileContext
from concourse.bass_utils import run_bass_kernel_spmd

ORDER = 3
B, D_IN, UNITS, NN = 4096, 64, 128, 36
N_CORES = 8
BS = B // N_CORES            # 512 batches per core
F32, BF16 = mybir.dt.float32, mybir.dt.bfloat16
F8E4, F8E5 = mybir.dt.float8e4, mybir.dt.float8e5

TBM = 36                     # batches per macro tile
GPK = 4                      # packs per transpose/conv group (12 batches)


# cc-index permutation putting the 768 conv input channels into six
# 128-partition chunks: [g1x|x], [h], [gh1], [g2x|g3x], [gh2], [gh3]
def _perm():
    r = lambda a, b: list(range(a, b))
    return (r(192, 256) + r(0, 64)      # C0: g1 x-rows + x (k0)
            + r(64, 192)                # H:  h (k0)
            + r(256, 384)               # GH1: g1 h-rows
            + r(384, 448) + r(576, 640)  # C3: g2 x-rows + g3 x-rows
            + r(448, 576)               # GH2
            + r(640, 768))              # GH3


def _chunks(W):
    WT = W.T[_perm(), :]                          # [768, 128]
    return WT.reshape(6, 128, 128)                # [chunk, cin, cout]


def _pack_wb(W):
    # bf16 chunks C0, H, C3 -> [128, 3*128] (cin on partition per chunk)
    ch = _chunks(W)
    sel = np.stack([ch[0], ch[1], ch[3]], axis=0)   # [3, 128, 128]
    return np.ascontiguousarray(
        sel.transpose(1, 0, 2).reshape(128, 384)).astype(ml_dtypes.bfloat16)


def _pack_w8(W):
    # fp8 DoubleRow pairs (GH1, GH2), (GH3, 0) -> [128, 4, 128] e5m2
    ch = _chunks(W)
    sel = np.stack([ch[2], ch[4], ch[5], np.zeros_like(ch[5])], axis=0)
    return np.ascontiguousarray(
        sel.transpose(1, 0, 2).reshape(128, 512)).astype(ml_dtypes.float8_e5m2)



def _mbd(adj, pack):
    # block-diag [pack*36, 3*pack*36]: cols (k, b_hat, w) of M^k = (adj.T)^k
    M1 = np.ascontiguousarray(adj.T).astype(np.float64)
    Ms = [M1, M1 @ M1, M1 @ M1 @ M1]
    P = pack * 36
    out = np.zeros((P, 3 * P), np.float64)
    for k in range(3):
        for bh in range(pack):
            out[bh * 36:(bh + 1) * 36, k * P + bh * 36:k * P + (bh + 1) * 36] = Ms[k]
    return out.astype(ml_dtypes.bfloat16)


def _build(bs):
    nc = bacc.Bacc("TRN2", target_bir_lowering=False, debug=False,
                   num_devices=N_CORES)
    d_x = nc.declare_dram_parameter("x", [bs, D_IN, NN], F32, isOutput=False)
    d_h = nc.declare_dram_parameter("h", [bs, UNITS, NN], F32, isOutput=False)
    dwb, dw8, dbias = {}, {}, {}
    for g in "fuc":
        dwb[g] = nc.declare_dram_parameter(f"w{g}b", [128, 384], BF16,
                                           isOutput=False)
        dw8[g] = nc.declare_dram_parameter(f"w{g}8", [128, 512], F8E5,
                                           isOutput=False)
        dbias[g] = nc.declare_dram_parameter(f"b{g}", [128, 1], F32,
                                             isOutput=False)
    d_m3 = nc.declare_dram_parameter("mbd3", [108, 324], BF16, isOutput=False)
    d_m2 = nc.declare_dram_parameter("mbd2", [72, 216], BF16, isOutput=False)
    d_id = nc.declare_dram_parameter("ident", [128, 128], BF16, isOutput=False)
    d_out = nc.declare_dram_parameter("out", [bs, UNITS, NN], F32, isOutput=True)

    SIG = mybir.ActivationFunctionType.Sigmoid
    TANH = mybir.ActivationFunctionType.Tanh
    DR = mybir.MatmulPerfMode.DoubleRow

    with TileContext(nc) as tc:
        with (
            tc.tile_pool(name="consts", bufs=1) as cpool,
            tc.tile_pool(name="macro", bufs=3) as mpool,
            tc.tile_pool(name="small", bufs=2) as spool,
            tc.tile_pool(name="ps_t", bufs=1, space="PSUM") as ps_t,
            tc.tile_pool(name="ps_g", bufs=2, space="PSUM") as ps_g,
            tc.tile_pool(name="ps_x", bufs=2, space="PSUM") as ps_x,
            tc.tile_pool(name="ps_conv", bufs=3, space="PSUM") as ps_conv,
        ):
            wb, w8, bias = {}, {}, {}
            for g in "fuc":
                wb[g] = cpool.tile([128, 384], BF16, name=f"w{g}b")
                w8[g] = cpool.tile([128, 4, 128], F8E5, name=f"w{g}8")
                bias[g] = cpool.tile([128, 1], F32, name=f"b{g}")
                nc.sync.dma_start(out=wb[g][:], in_=dwb[g][:])
                nc.sync.dma_start(
                    out=w8[g][:].rearrange("p j m -> p (j m)"), in_=dw8[g][:])
                nc.sync.dma_start(out=bias[g][:], in_=dbias[g][:])
            m3 = cpool.tile([108, 324], BF16, name="m3")
            m2 = cpool.tile([72, 216], BF16, name="m2")
            ident = cpool.tile([128, 128], BF16, name="ident")
            for dst, src in ((m3, d_m3), (m2, d_m2), (ident, d_id)):
                nc.sync.dma_start(out=dst[:], in_=src[:])
            id_hi = ident[64:128, 64:128]

            macros = [(0, 12)]
            while macros[-1][0] + macros[-1][1] < bs:
                s = macros[-1][0] + macros[-1][1]
                macros.append((s, min(TBM, bs - s)))

            for b0, nb in macros:
                F = nb * 36
                packs = [3] * (nb // 3)
                if nb % 3 == 2:
                    packs.append(2)
                elif nb % 3 == 1:
                    packs[-1] = 2
                    packs.append(2)
                gstarts = []
                col = 0
                for p in packs:
                    gstarts.append((col, p))
                    col += p * 36
                groups = [gstarts[i:i + GPK] for i in range(0, len(gstarts), GPK)]

                c0 = mpool.tile([128, F], BF16, tag="c0", name=f"c0_{b0}")
                xx32 = mpool.tile([64, F], F32, tag="xx32", name=f"xx32_{b0}")
                hh = mpool.tile([128, F], F32, tag="hh", name=f"hh_{b0}")
                hh16 = mpool.tile([128, F], BF16, tag="hh16", name=f"hh16_{b0}")
                c3 = mpool.tile([128, F], BF16, tag="c3", name=f"c3_{b0}")
                ghg = mpool.tile([128, 4, F], F8E4, tag="ghg", name=f"ghg_{b0}")
                rr16 = mpool.tile([128, F], BF16, tag="rr16", name=f"rr16_{b0}")
                uu16 = mpool.tile([128, F], BF16, tag="uu16", name=f"uu16_{b0}")
                rhh16 = mpool.tile([128, F], BF16, tag="rhh16",
                                   name=f"rhh16_{b0}")
                rhg = mpool.tile([128, 4, F], F8E4, tag="rhg", name=f"rhg_{b0}")
                ct16 = mpool.tile([128, F], BF16, tag="ct16", name=f"ct16_{b0}")
                oo = mpool.tile([128, F], F32, tag="oo", name=f"oo_{b0}")

                nc.vector.memzero(ghg[:, 3, :])
                nc.vector.memzero(rhg[:, 3, :])

                gcols = []
                col = 0
                for grp in groups:
                    W = sum(p for _, p in grp) * 36
                    gcols.append(slice(col, col + W))
                    col += W

                # --- diffusion of z = [x; h]: per group of <=4 packs,
                # batched same-shape PE transposes (layout [h | x] per pack),
                # one copy to SBUF, then per-pack block-diag node-mixes ---
                def zphase(gi):
                    grp = groups[gi]
                    gs = gcols[gi]
                    bgsl = slice(b0 + gs.start // 36, b0 + gs.stop // 36)
                    nbg = bgsl.stop - bgsl.start
                    nc.sync.dma_start(
                        out=xx32[:, gs].rearrange("c (b n) -> c b n", b=nbg),
                        in_=d_x[bgsl].rearrange("b c n -> c b n"))
                    nc.sync.dma_start(
                        out=hh[:, gs].rearrange("c (b n) -> c b n", b=nbg),
                        in_=d_h[bgsl].rearrange("b c n -> c b n"))
                    nc.vector.tensor_copy(c0[64:128, gs], xx32[:, gs])
                    nc.vector.tensor_copy(hh16[:, gs], hh[:, gs])
                    ztp = ps_t.tile([108, 768], BF16, tag="ztp",
                                    name=f"ztp_{b0}_{gi}")
                    zts = spool.tile([108, 768], BF16, tag="zts",
                                     name=f"zts_{b0}_{gi}")
                    for j, (g0, p) in enumerate(grp):
                        P = p * 36
                        nc.tensor.transpose(ztp[0:P, j * 192:j * 192 + 128],
                                            hh16[:, g0:g0 + P], ident[:])
                    for j, (g0, p) in enumerate(grp):
                        P = p * 36
                        nc.tensor.transpose(
                            ztp[0:P, j * 192 + 128:j * 192 + 192],
                            c0[64:128, g0:g0 + P], id_hi)
                    nc.vector.tensor_copy(zts[:, 0:len(grp) * 192],
                                          ztp[:, 0:len(grp) * 192])
                    # per-pair emission: h-mix + x-k1k2 for both packs, then
                    # the two col-group-switch k3 matmuls back to back (one
                    # pipeline stall instead of two), then the copies
                    jpairs = [list(enumerate(grp))[i:i + 2]
                              for i in range(0, len(grp), 2)]
                    for pair in jpairs:
                        pgxs = {}
                        for j, (g0, p) in pair:
                            P = p * 36
                            mbd = m3 if p == 3 else m2
                            zo = j * 192
                            pgh = ps_g.tile([128, 324], F32, tag="pgh",
                                            name=f"pgh_{b0}_{gi}_{j}")
                            pgx = ps_x.tile([128, 216], F32, tag="pgx",
                                            name=f"pgx_{b0}_{gi}_{j}")
                            pgxs[j] = (pgh, pgx)
                            nc.tensor.matmul(pgh[:, 0:3 * P],
                                             zts[0:P, zo:zo + 128], mbd[:])
                            nc.tensor.matmul(pgx[0:64, 0:2 * P],
                                             zts[0:P, zo + 128:zo + 192],
                                             mbd[:, 0:2 * P])
                        for j, (g0, p) in pair:
                            P = p * 36
                            mbd = m3 if p == 3 else m2
                            zo = j * 192
                            nc.tensor.matmul(pgxs[j][1][64:128, P:2 * P],
                                             zts[0:P, zo + 128:zo + 192],
                                             mbd[:, 2 * P:3 * P])
                        # drain pgx first (c0/c3 ahead of the big ghg
                        # copies in the engine queues) so the x-mix psum
                        # recycles without stalling the next pack's k3
                        for j, (g0, p) in pair:
                            P = p * 36
                            nc.scalar.copy(c0[0:64, g0:g0 + P],
                                           pgxs[j][1][0:64, 0:P])
                            nc.vector.tensor_copy(c3[:, g0:g0 + P],
                                                  pgxs[j][1][:, P:2 * P])
                        for j, (g0, p) in pair:
                            P = p * 36
                            nc.scalar.copy(
                                ghg[:, 0:3, g0:g0 + P],
                                pgxs[j][0][:, 0:3 * P].rearrange(
                                    "c (k w) -> c k w", k=3))

                tiles = gcols
                ngrp = len(groups)

                def conv(g, z0, z1, gt, t, psname):
                    pc = ps_conv.tile([128, 432], F32, tag="pconv",
                                      name=psname)
                    T = t.stop - t.start
                    w_sb, w_f8 = wb[g], w8[g]
                    nc.tensor.matmul(pc[:, 0:T], w_sb[:, 0:128], z0[:, t],
                                     start=True, stop=False)
                    nc.tensor.matmul(pc[:, 0:T], w_sb[:, 128:256], z1[:, t],
                                     start=False, stop=False)
                    nc.tensor.matmul(pc[:, 0:T], w_sb[:, 256:384], c3[:, t],
                                     start=False, stop=False)
                    nc.tensor.matmul(pc[:, 0:T], w_f8[:, 0:2, :],
                                     gt[:, 0:2, t], start=False, stop=False,
                                     perf_mode=DR)
                    nc.tensor.matmul(pc[:, 0:T], w_f8[:, 2:4, :],
                                     gt[:, 2:4, t], start=False, stop=True,
                                     perf_mode=DR)
                    return pc

                def fphase(ti):
                    t = tiles[ti]
                    pc = conv("f", c0, hh16, ghg, t, f"pcf_{b0}_{t.start}")
                    nc.scalar.activation(rr16[:, t], pc[:, 0:t.stop - t.start],
                                         SIG, bias=bias["f"][:, 0:1])
                    nc.gpsimd.tensor_mul(rhh16[:, t], rr16[:, t], hh16[:, t])

                def uphase(ti):
                    t = tiles[ti]
                    pc = conv("u", c0, hh16, ghg, t, f"pcu_{b0}_{t.start}")
                    nc.scalar.activation(uu16[:, t], pc[:, 0:t.stop - t.start],
                                         SIG, bias=bias["u"][:, 0:1])

                # --- diffusion of r*h per group ---
                def rphase(gi):
                    grp = groups[gi]
                    ztp = ps_t.tile([108, 768], BF16, tag="ztp",
                                    name=f"ztr_{b0}_{gi}")
                    zts = spool.tile([108, 768], BF16, tag="zts",
                                     name=f"ztsr_{b0}_{gi}")
                    for j, (g0, p) in enumerate(grp):
                        P = p * 36
                        nc.tensor.transpose(ztp[0:P, j * 128:j * 128 + 128],
                                            rhh16[:, g0:g0 + P], ident[:])
                    nc.vector.tensor_copy(zts[:, 0:len(grp) * 128],
                                          ztp[:, 0:len(grp) * 128])
                    for j, (g0, p) in enumerate(grp):
                        P = p * 36
                        mbd = m3 if p == 3 else m2
                        prh = ps_g.tile([128, 324], F32, tag="pgh",
                                        name=f"prh_{b0}_{gi}_{j}")
                        nc.tensor.matmul(prh[:, 0:3 * P],
                                         zts[0:P, j * 128:j * 128 + 128],
                                         mbd[:])
                        nc.vector.tensor_copy(
                            rhg[:, 0:3, g0:g0 + P],
                            prh[:, 0:3 * P].rearrange("c (k w) -> c k w", k=3))

                # --- candidate gate + output (out = c + u * (h - c)) ---
                def cphase(ti):
                    t = tiles[ti]
                    pc = conv("c", c0, rhh16, rhg, t, f"pcc_{b0}_{t.start}")
                    nc.scalar.activation(ct16[:, t], pc[:, 0:t.stop - t.start],
                                         TANH, bias=bias["c"][:, 0:1])
                    nc.gpsimd.tensor_sub(oo[:, t], hh[:, t], ct16[:, t])
                    nc.gpsimd.tensor_mul(oo[:, t], oo[:, t], uu16[:, t])
                    nc.gpsimd.tensor_add(oo[:, t], oo[:, t], ct16[:, t])
                    bgsl = slice(b0 + t.start // 36, b0 + t.stop // 36)
                    nbg = bgsl.stop - bgsl.start
                    nc.sync.dma_start(
                        out=d_out[bgsl].rearrange("b c n -> c b n"),
                        in_=oo[:, t].rearrange("c (b n) -> c b n", b=nbg))

                # interleave diffusion groups with conv tiles in program
                # order so the PE never sees a long transpose/mix desert
                # (keeps the HAM clock gate warm): z0 z1 f0 z2 f1 f2 u0 r0
                # u1 r1 u2 r2 c0 c1 c2
                zphase(0)
                if ngrp > 1:
                    zphase(1)
                for i in range(ngrp):
                    fphase(i)
                    if i + 2 < ngrp:
                        zphase(i + 2)
                for i in range(ngrp):
                    uphase(i)
                    rphase(i)
                for i in range(ngrp):
                    cphase(i)
    nc.compile()
    return nc


_CACHE = {}
LAST_RESULTS = None


def _get_nc(bs):
    if bs not in _CACHE:
        _CACHE[bs] = _build(bs)
    return _CACHE[bs]


def kernel(x, h, adj, W_f, b_f, W_u, b_u, W_c, b_c):
    x = np.ascontiguousarray(x, np.float32)
    h = np.ascontiguousarray(h, np.float32)
    consts = {
        "wfb": _pack_wb(np.asarray(W_f, np.float32)),
        "wub": _pack_wb(np.asarray(W_u, np.float32)),
        "wcb": _pack_wb(np.asarray(W_c, np.float32)),
        "wf8": _pack_w8(np.asarray(W_f, np.float32)),
        "wu8": _pack_w8(np.asarray(W_u, np.float32)),
        "wc8": _pack_w8(np.asarray(W_c, np.float32)),
        "bf": np.asarray(b_f, np.float32).reshape(128, 1),
        "bu": np.asarray(b_u, np.float32).reshape(128, 1),
        "bc": np.asarray(b_c, np.float32).reshape(128, 1),
        "mbd3": _mbd(np.asarray(adj, np.float32), 3),
        "mbd2": _mbd(np.asarray(adj, np.float32), 2),
        "ident": np.eye(128, dtype=ml_dtypes.bfloat16),
    }
    bs = x.shape[0] // N_CORES
    nc = _get_nc(bs)
    in_maps = [
        {"x": x[i * bs:(i + 1) * bs], "h": h[i * bs:(i + 1) * bs], **consts}
        for i in range(N_CORES)
    ]
    res = run_bass_kernel_spmd(nc, in_maps, list(range(N_CORES)))
    global LAST_RESULTS
    LAST_RESULTS = res
    return np.concatenate([res.results[i]["out"] for i in range(N_CORES)], axis=0)


# revision 37
# speedup vs baseline: 1.0421x; 1.0421x over previous
"""GCGRU cell (graph-conv GRU, diffusion order 3) on 8 TRN2 NeuronCores.

Data-parallel over the batch dim (512 per core). Per core, activations are
channel-on-partition [C, (b, n)]; the node-dim diffusion transposes 3-batch
groups through the PE transpose datapath (batched same-shape so they
pipeline) and multiplies against a host-precomputed block-diagonal
[M^1 | M^2 | M^3]. The h-diffusion conv chunks (gh1..gh3, ~1% of the
pre-activation variance) run as fp8 DoubleRow matmuls (e4m3 activations x
e5m2 weights, scales 1 so they accumulate straight into the shared f32 PSUM
group with the bf16 chunks); the z chunks stay bf16. sigmoid/tanh run on the
scalar engine out of PSUM with fused bias; copies are spread across the
scalar/vector/gpsimd engines.
"""
import numpy as np
import ml_dtypes

import concourse.bacc as bacc
import concourse.mybir as mybir
from concourse.tile import TileContext
from concourse.bass_utils import run_bass_kernel_spmd

ORDER = 3
B, D_IN, UNITS, NN = 4096, 64, 128, 36
N_CORES = 8
BS = B // N_CORES            # 512 batches per core
F32, BF16 = mybir.dt.float32, mybir.dt.bfloat16
F8E4, F8E5 = mybir.dt.float8e4, mybir.dt.float8e5

TBM = 48                     # batches per macro tile
GPK = 4                      # packs per transpose/conv group (12 batches)


# cc-index permutation putting the 768 conv input channels into six
# 128-partition chunks: [g1x|x], [h], [gh1], [g2x|g3x], [gh2], [gh3]
def _perm():
    r = lambda a, b: list(range(a, b))
    return (r(192, 256) + r(0, 64)      # C0: g1 x-rows + x (k0)
            + r(64, 192)                # H:  h (k0)
            + r(256, 384)               # GH1: g1 h-rows
            + r(384, 448) + r(576, 640)  # C3: g2 x-rows + g3 x-rows
            + r(448, 576)               # GH2
            + r(640, 768))              # GH3


def _chunks(W):
    WT = W.T[_perm(), :]                          # [768, 128]
    return WT.reshape(6, 128, 128)                # [chunk, cin, cout]


def _pack_wb(W):
    # bf16 chunks C0, H, C3 -> [128, 3*128] (cin on partition per chunk)
    ch = _chunks(W)
    sel = np.stack([ch[0], ch[1], ch[3]], axis=0)   # [3, 128, 128]
    return np.ascontiguousarray(
        sel.transpose(1, 0, 2).reshape(128, 384)).astype(ml_dtypes.bfloat16)


def _pack_w8(W):
    # fp8 DoubleRow pairs (GH1, GH2), (GH3, 0) -> [128, 4, 128] e5m2
    ch = _chunks(W)
    sel = np.stack([ch[2], ch[4], ch[5], np.zeros_like(ch[5])], axis=0)
    return np.ascontiguousarray(
        sel.transpose(1, 0, 2).reshape(128, 512)).astype(ml_dtypes.float8_e5m2)



def _mbd(adj, pack):
    # block-diag [pack*36, 3*pack*36]: cols (k, b_hat, w) of M^k = (adj.T)^k
    M1 = np.ascontiguousarray(adj.T).astype(np.float64)
    Ms = [M1, M1 @ M1, M1 @ M1 @ M1]
    P = pack * 36
    out = np.zeros((P, 3 * P), np.float64)
    for k in range(3):
        for bh in range(pack):
            out[bh * 36:(bh + 1) * 36, k * P + bh * 36:k * P + (bh + 1) * 36] = Ms[k]
    return out.astype(ml_dtypes.bfloat16)


def _build(bs):
    nc = bacc.Bacc("TRN2", target_bir_lowering=False, debug=False,
                   num_devices=N_CORES)
    d_x = nc.declare_dram_parameter("x", [bs, D_IN, NN], F32, isOutput=False)
    d_h = nc.declare_dram_parameter("h", [bs, UNITS, NN], F32, isOutput=False)
    dwb, dw8, dbias = {}, {}, {}
    for g in "fuc":
        dwb[g] = nc.declare_dram_parameter(f"w{g}b", [128, 384], BF16,
                                           isOutput=False)
        dw8[g] = nc.declare_dram_parameter(f"w{g}8", [128, 512], F8E5,
                                           isOutput=False)
        dbias[g] = nc.declare_dram_parameter(f"b{g}", [128, 1], F32,
                                             isOutput=False)
    d_m3 = nc.declare_dram_parameter("mbd3", [108, 324], BF16, isOutput=False)
    d_m2 = nc.declare_dram_parameter("mbd2", [72, 216], BF16, isOutput=False)
    d_id = nc.declare_dram_parameter("ident", [128, 128], BF16, isOutput=False)
    d_out = nc.declare_dram_parameter("out", [bs, UNITS, NN], F32, isOutput=True)

    SIG = mybir.ActivationFunctionType.Sigmoid
    TANH = mybir.ActivationFunctionType.Tanh
    DR = mybir.MatmulPerfMode.DoubleRow

    with TileContext(nc) as tc:
        with (
            tc.tile_pool(name="consts", bufs=1) as cpool,
            tc.tile_pool(name="macro", bufs=2) as mpool,
            tc.tile_pool(name="small", bufs=2) as spool,
            tc.tile_pool(name="ps_t", bufs=1, space="PSUM") as ps_t,
            tc.tile_pool(name="ps_g", bufs=2, space="PSUM") as ps_g,
            tc.tile_pool(name="ps_x", bufs=3, space="PSUM") as ps_x,
            tc.tile_pool(name="ps_conv", bufs=2, space="PSUM") as ps_conv,
        ):
            wb, w8, bias = {}, {}, {}
            for g in "fuc":
                wb[g] = cpool.tile([128, 384], BF16, name=f"w{g}b")
                w8[g] = cpool.tile([128, 4, 128], F8E5, name=f"w{g}8")
                bias[g] = cpool.tile([128, 1], F32, name=f"b{g}")
                nc.sync.dma_start(out=wb[g][:], in_=dwb[g][:])
                nc.sync.dma_start(
                    out=w8[g][:].rearrange("p j m -> p (j m)"), in_=dw8[g][:])
                nc.sync.dma_start(out=bias[g][:], in_=dbias[g][:])
            m3 = cpool.tile([108, 324], BF16, name="m3")
            m2 = cpool.tile([72, 216], BF16, name="m2")
            ident = cpool.tile([128, 128], BF16, name="ident")
            for dst, src in ((m3, d_m3), (m2, d_m2), (ident, d_id)):
                nc.sync.dma_start(out=dst[:], in_=src[:])
            id_hi = ident[64:128, 64:128]

            macros = [(0, 12)]
            while macros[-1][0] + macros[-1][1] < bs:
                s = macros[-1][0] + macros[-1][1]
                macros.append((s, min(TBM, bs - s)))

            for b0, nb in macros:
                F = nb * 36
                packs = [3] * (nb // 3)
                if nb % 3 == 2:
                    packs.append(2)
                elif nb % 3 == 1:
                    packs[-1] = 2
                    packs.append(2)
                gstarts = []
                col = 0
                for p in packs:
                    gstarts.append((col, p))
                    col += p * 36
                groups = [gstarts[i:i + GPK] for i in range(0, len(gstarts), GPK)]

                c0 = mpool.tile([128, F], BF16, tag="c0", name=f"c0_{b0}")
                xx32 = mpool.tile([64, F], F32, tag="xx32", name=f"xx32_{b0}")
                hh = mpool.tile([128, F], F32, tag="hh", name=f"hh_{b0}")
                hh16 = mpool.tile([128, F], BF16, tag="hh16", name=f"hh16_{b0}")
                c3 = mpool.tile([128, F], BF16, tag="c3", name=f"c3_{b0}")
                ghg = mpool.tile([128, 4, F], F8E4, tag="ghg", name=f"ghg_{b0}")
                rr16 = mpool.tile([128, F], BF16, tag="rr16", name=f"rr16_{b0}")
                uu16 = mpool.tile([128, F], BF16, tag="uu16", name=f"uu16_{b0}")
                rhh16 = mpool.tile([128, F], BF16, tag="rhh16",
                                   name=f"rhh16_{b0}")
                rhg = mpool.tile([128, 4, F], F8E4, tag="rhg", name=f"rhg_{b0}")
                ct16 = mpool.tile([128, F], BF16, tag="ct16", name=f"ct16_{b0}")
                oo = mpool.tile([128, F], F32, tag="oo", name=f"oo_{b0}")

                nc.vector.memzero(ghg[:, 3, :])
                nc.vector.memzero(rhg[:, 3, :])

                gcols = []
                col = 0
                for grp in groups:
                    W = sum(p for _, p in grp) * 36
                    gcols.append(slice(col, col + W))
                    col += W

                # --- diffusion of z = [x; h]: per group of <=4 packs,
                # batched same-shape PE transposes (layout [h | x] per pack),
                # one copy to SBUF, then per-pack block-diag node-mixes ---
                def zphase(gi):
                    grp = groups[gi]
                    gs = gcols[gi]
                    bgsl = slice(b0 + gs.start // 36, b0 + gs.stop // 36)
                    nbg = bgsl.stop - bgsl.start
                    nc.sync.dma_start(
                        out=xx32[:, gs].rearrange("c (b n) -> c b n", b=nbg),
                        in_=d_x[bgsl].rearrange("b c n -> c b n"))
                    nc.sync.dma_start(
                        out=hh[:, gs].rearrange("c (b n) -> c b n", b=nbg),
                        in_=d_h[bgsl].rearrange("b c n -> c b n"))
                    nc.vector.tensor_copy(c0[64:128, gs], xx32[:, gs])
                    nc.vector.tensor_copy(hh16[:, gs], hh[:, gs])
                    ztp = ps_t.tile([108, 768], BF16, tag="ztp",
                                    name=f"ztp_{b0}_{gi}")
                    zts = spool.tile([108, 768], BF16, tag="zts",
                                     name=f"zts_{b0}_{gi}")
                    for j, (g0, p) in enumerate(grp):
                        P = p * 36
                        nc.tensor.transpose(ztp[0:P, j * 192:j * 192 + 128],
                                            hh16[:, g0:g0 + P], ident[:])
                    for j, (g0, p) in enumerate(grp):
                        P = p * 36
                        nc.tensor.transpose(
                            ztp[0:P, j * 192 + 128:j * 192 + 192],
                            c0[64:128, g0:g0 + P], id_hi)
                    nc.vector.tensor_copy(zts[:, 0:len(grp) * 192],
                                          ztp[:, 0:len(grp) * 192])
                    # per-pair emission: h-mix + x-k1k2 for both packs, then
                    # the two col-group-switch k3 matmuls back to back (one
                    # pipeline stall instead of two), then the copies
                    jpairs = [list(enumerate(grp))[i:i + 2]
                              for i in range(0, len(grp), 2)]
                    for pair in jpairs:
                        pgxs = {}
                        for j, (g0, p) in pair:
                            P = p * 36
                            mbd = m3 if p == 3 else m2
                            zo = j * 192
                            pgh = ps_g.tile([128, 324], F32, tag="pgh",
                                            name=f"pgh_{b0}_{gi}_{j}")
                            pgx = ps_x.tile([128, 216], F32, tag="pgx",
                                            name=f"pgx_{b0}_{gi}_{j}")
                            pgxs[j] = (pgh, pgx)
                            nc.tensor.matmul(pgh[:, 0:3 * P],
                                             zts[0:P, zo:zo + 128], mbd[:])
                            nc.tensor.matmul(pgx[0:64, 0:2 * P],
                                             zts[0:P, zo + 128:zo + 192],
                                             mbd[:, 0:2 * P])
                        for j, (g0, p) in pair:
                            P = p * 36
                            mbd = m3 if p == 3 else m2
                            zo = j * 192
                            nc.tensor.matmul(pgxs[j][1][64:128, P:2 * P],
                                             zts[0:P, zo + 128:zo + 192],
                                             mbd[:, 2 * P:3 * P])
                        # drain pgx first (c0/c3 ahead of the big ghg
                        # copies in the engine queues) so the x-mix psum
                        # recycles without stalling the next pack's k3
                        for j, (g0, p) in pair:
                            P = p * 36
                            nc.scalar.copy(c0[0:64, g0:g0 + P],
                                           pgxs[j][1][0:64, 0:P])
                            nc.vector.tensor_copy(c3[:, g0:g0 + P],
                                                  pgxs[j][1][:, P:2 * P])
                        for j, (g0, p) in pair:
                            P = p * 36
                            nc.scalar.copy(
                                ghg[:, 0:3, g0:g0 + P],
                                pgxs[j][0][:, 0:3 * P].rearrange(
                                    "c (k w) -> c k w", k=3))

                tiles = gcols
                ngrp = len(groups)

                def conv(g, z0, z1, gt, t, psname):
                    pc = ps_conv.tile([128, 432], F32, tag="pconv",
                                      name=psname)
                    T = t.stop - t.start
                    w_sb, w_f8 = wb[g], w8[g]
                    nc.tensor.matmul(pc[:, 0:T], w_sb[:, 0:128], z0[:, t],
                                     start=True, stop=False)
                    nc.tensor.matmul(pc[:, 0:T], w_sb[:, 128:256], z1[:, t],
                                     start=False, stop=False)
                    nc.tensor.matmul(pc[:, 0:T], w_sb[:, 256:384], c3[:, t],
                                     start=False, stop=False)
                    nc.tensor.matmul(pc[:, 0:T], w_f8[:, 0:2, :],
                                     gt[:, 0:2, t], start=False, stop=False,
                                     perf_mode=DR)
                    nc.tensor.matmul(pc[:, 0:T], w_f8[:, 2:4, :],
                                     gt[:, 2:4, t], start=False, stop=True,
                                     perf_mode=DR)
                    return pc

                def fphase(ti):
                    t = tiles[ti]
                    pc = conv("f", c0, hh16, ghg, t, f"pcf_{b0}_{t.start}")
                    nc.scalar.activation(rr16[:, t], pc[:, 0:t.stop - t.start],
                                         SIG, bias=bias["f"][:, 0:1])
                    nc.gpsimd.tensor_mul(rhh16[:, t], rr16[:, t], hh16[:, t])

                def uphase(ti):
                    t = tiles[ti]
                    pc = conv("u", c0, hh16, ghg, t, f"pcu_{b0}_{t.start}")
                    nc.scalar.activation(uu16[:, t], pc[:, 0:t.stop - t.start],
                                         SIG, bias=bias["u"][:, 0:1])

                # --- diffusion of r*h per group ---
                def rphase(gi):
                    grp = groups[gi]
                    ztp = ps_t.tile([108, 768], BF16, tag="ztp",
                                    name=f"ztr_{b0}_{gi}")
                    zts = spool.tile([108, 768], BF16, tag="zts",
                                     name=f"ztsr_{b0}_{gi}")
                    for j, (g0, p) in enumerate(grp):
                        P = p * 36
                        nc.tensor.transpose(ztp[0:P, j * 128:j * 128 + 128],
                                            rhh16[:, g0:g0 + P], ident[:])
                    nc.vector.tensor_copy(zts[:, 0:len(grp) * 128],
                                          ztp[:, 0:len(grp) * 128])
                    for j, (g0, p) in enumerate(grp):
                        P = p * 36
                        mbd = m3 if p == 3 else m2
                        prh = ps_g.tile([128, 324], F32, tag="pgh",
                                        name=f"prh_{b0}_{gi}_{j}")
                        nc.tensor.matmul(prh[:, 0:3 * P],
                                         zts[0:P, j * 128:j * 128 + 128],
                                         mbd[:])
                        nc.vector.tensor_copy(
                            rhg[:, 0:3, g0:g0 + P],
                            prh[:, 0:3 * P].rearrange("c (k w) -> c k w", k=3))

                # --- candidate gate + output (out = c + u * (h - c)) ---
                def cphase(ti):
                    t = tiles[ti]
                    pc = conv("c", c0, rhh16, rhg, t, f"pcc_{b0}_{t.start}")
                    nc.scalar.activation(ct16[:, t], pc[:, 0:t.stop - t.start],
                                         TANH, bias=bias["c"][:, 0:1])
                    nc.gpsimd.tensor_sub(oo[:, t], hh[:, t], ct16[:, t])
                    nc.gpsimd.tensor_mul(oo[:, t], oo[:, t], uu16[:, t])
                    nc.gpsimd.tensor_add(oo[:, t], oo[:, t], ct16[:, t])
                    bgsl = slice(b0 + t.start // 36, b0 + t.stop // 36)
                    nbg = bgsl.stop - bgsl.start
                    nc.sync.dma_start(
                        out=d_out[bgsl].rearrange("b c n -> c b n"),
                        in_=oo[:, t].rearrange("c (b n) -> c b n", b=nbg))

                # interleave diffusion groups with conv tiles in program
                # order so the PE never sees a long transpose/mix desert
                # (keeps the HAM clock gate warm): z0 z1 f0 z2 f1 f2 u0 r0
                # u1 r1 u2 r2 c0 c1 c2
                zphase(0)
                if ngrp > 1:
                    zphase(1)
                for i in range(ngrp):
                    fphase(i)
                    if i + 2 < ngrp:
                        zphase(i + 2)
                for i in range(ngrp):
                    uphase(i)
                    rphase(i)
                for i in range(ngrp):
                    cphase(i)
    nc.compile()
    return nc


_CACHE = {}
LAST_RESULTS = None


def _get_nc(bs):
    if bs not in _CACHE:
        _CACHE[bs] = _build(bs)
    return _CACHE[bs]


def kernel(x, h, adj, W_f, b_f, W_u, b_u, W_c, b_c):
    x = np.ascontiguousarray(x, np.float32)
    h = np.ascontiguousarray(h, np.float32)
    consts = {
        "wfb": _pack_wb(np.asarray(W_f, np.float32)),
        "wub": _pack_wb(np.asarray(W_u, np.float32)),
        "wcb": _pack_wb(np.asarray(W_c, np.float32)),
        "wf8": _pack_w8(np.asarray(W_f, np.float32)),
        "wu8": _pack_w8(np.asarray(W_u, np.float32)),
        "wc8": _pack_w8(np.asarray(W_c, np.float32)),
        "bf": np.asarray(b_f, np.float32).reshape(128, 1),
        "bu": np.asarray(b_u, np.float32).reshape(128, 1),
        "bc": np.asarray(b_c, np.float32).reshape(128, 1),
        "mbd3": _mbd(np.asarray(adj, np.float32), 3),
        "mbd2": _mbd(np.asarray(adj, np.float32), 2),
        "ident": np.eye(128, dtype=ml_dtypes.bfloat16),
    }
    bs = x.shape[0] // N_CORES
    nc = _get_nc(bs)
    in_maps = [
        {"x": x[i * bs:(i + 1) * bs], "h": h[i * bs:(i + 1) * bs], **consts}
        for i in range(N_CORES)
    ]
    res = run_bass_kernel_spmd(nc, in_maps, list(range(N_CORES)))
    global LAST_RESULTS
    LAST_RESULTS = res
    return np.concatenate([res.results[i]["out"] for i in range(N_CORES)], axis=0)
